# revision 6
# baseline (speedup 1.0000x reference)
"""APPNP (MLP + K-step personalized-PageRank propagation) on 8 trn2 NeuronCores.

Strategy (node-sharded):
  - 8 cores each own 12500 destination nodes (contiguous global ranges),
    re-binned on host into 98 tiles of <=128 dests equalizing per-bucket
    in-edge counts.
  - Full feature table p = dinv * h, [8*12544, 64] f32, replicated per core
    via AllGather each step (internal shared DRAM).
  - Per step each core gathers its in-edge source rows with dma_gather
    (4 SWDGE queues, int16 indices bucketed into 4 source windows),
    segment-sums via one-hot selection matmuls into PSUM per dest tile,
    applies (1-a)*dinv*agg + a*x0, writes p back, AllGather.
  - GCN norm factorized: norm = dinv[row]*dinv[col]; dinv[row] is folded
    into the gathered table (p = dinv*h), dinv[col] applied at the epilogue.
"""

import contextlib
import os

import numpy as np

import concourse.bacc as bacc
import concourse.bass as bass
import concourse.mybir as mybir
import concourse.tile as tile
from concourse import bass_utils
from concourse.library_config import mlp as mlp_lib

N_NODES = 100000
N_EDGES = 1600000
IN_DIM, HIDDEN, EMBED = 128, 256, 64
K_STEPS = int(os.environ.get("APPNP_KSTEPS", "10"))
ALPHA = 0.1

W = 8                      # cores
PER = N_NODES // W         # 12500 dests per core
NT = 98                    # dest tiles (bins) per core
ROWS = NT * 128            # 12544 padded shard rows
NBUK = 4                   # source windows (int16 range)
WIN = ROWS * W // NBUK     # 25088 table rows per window

LAST_EXEC_TIME_NS = None


# ----------------------------------------------------------------- host prep

def _pack_bins(deg_b, cap_cell):
    """Assign 12500 dests to 98 bins (<=128 each), equalizing per-bucket
    edge loads. deg_b: [12500, 4] per-bucket in-degree (incl self loop).
    Tries to keep every (bin, bucket) load <= cap_cell."""
    n = deg_b.shape[0]
    order = np.argsort(-deg_b.max(1), kind="stable")
    loads = np.zeros((NT, NBUK), np.int64)
    counts = np.zeros(NT, np.int64)
    assign = np.empty(n, np.int64)
    for d in order:
        cand = np.flatnonzero(counts < 128)
        nl = loads[cand] + deg_b[d]
        over = np.maximum(nl - cap_cell, 0).sum(1)
        sc = over * 10_000_000 + nl.max(1) * 1000 + counts[cand]
        i = cand[np.argmin(sc)]
        assign[d] = i
        loads[i] += deg_b[d]
        counts[i] += 1
    # repair: move dests out of over-cap cells when possible
    for _ in range(3):
        bad = np.argwhere(loads > cap_cell)
        if len(bad) == 0:
            break
        for t, b in bad:
            if loads[t, b] <= cap_cell:
                continue
            members = np.flatnonzero(assign == t)
            members = members[np.argsort(-deg_b[members, b])]
            for d in members:
                if loads[t, b] <= cap_cell:
                    break
                room = np.flatnonzero(counts < 128)
                nl = loads[room] + deg_b[d]
                ok = room[(nl <= cap_cell).all(1) & (room != t)]
                if len(ok):
                    nt2 = ok[np.argmin((loads[ok] + deg_b[d]).max(1))]
                    assign[d] = nt2
                    loads[t] -= deg_b[d]
                    loads[nt2] += deg_b[d]
                    counts[t] -= 1
                    counts[nt2] += 1
    return assign, loads


def _prep(edge_index):
    row = np.asarray(edge_index[0], np.int64)
    col = np.asarray(edge_index[1], np.int64)

    deg = np.bincount(col, minlength=N_NODES).astype(np.float64) + 1.0
    dinv = (1.0 / np.sqrt(deg)).astype(np.float32)

    src_core = row // PER
    dst_core = col // PER
    loops = np.arange(N_NODES, dtype=np.int64)

    # ---- phase 1: per-core binning -> global table_row
    tile_of = np.empty(N_NODES, np.int64)
    pos_of = np.empty(N_NODES, np.int64)
    max_cell = 0
    for c in range(W):
        lo, hi = c * PER, (c + 1) * PER
        m = dst_core == c
        er, ec = row[m], col[m] - lo
        # bucket of a source = its core pair (window layout == core pairs)
        eb = (er // PER) // 2
        deg_b = np.zeros((PER, NBUK), np.int64)
        np.add.at(deg_b, (ec, eb), 1)
        assign, loads = _pack_bins(deg_b, cap_cell=5 * 128)
        max_cell = max(max_cell, int(loads.max()))
        tile_of[lo:hi] = assign
        # position within bin: stable order by local id
        order = np.lexsort((np.arange(PER), assign))
        pos = np.empty(PER, np.int64)
        # rank within each bin
        binstart = {}
        k = 0
        prev = -1
        for ld in order:
            b = assign[ld]
            if b != prev:
                k = 0
                prev = b
            pos[ld] = k
            k += 1
        pos_of[lo:hi] = pos

    C = int(-(-max_cell // 128))  # chunks per (tile, bucket)
    table_row = (col * 0)  # placeholder
    table_row = 12544 * (loops // PER) + 128 * tile_of + pos_of  # [N]

    # ---- phase 2: per-core gather/selection streams
    cores = []
    for c in range(W):
        lo = c * PER
        m = dst_core == c
        er, ec = row[m], col[m]
        et = tile_of[ec]
        ep = pos_of[ec]
        etr = table_row[er]
        eb = etr // WIN
        ei = etr - eb * WIN
        # order by (tile, bucket)
        o = np.lexsort((eb, et))
        et, eb, ei, ep = et[o], eb[o], ei[o], ep[o]
        key = et * NBUK + eb
        nslot = NT * NBUK * C * 128
        idx = np.zeros(NT * NBUK * C * 128, np.int16)
        dlc = np.full(NT * NBUK * C * 128, -1.0, np.float32)
        starts = np.searchsorted(key, np.arange(NT * NBUK))
        ends = np.searchsorted(key, np.arange(NT * NBUK) + 1)
        cnt = ends - starts
        assert cnt.max() <= C * 128, (cnt.max(), C)
        # destination slot for edge j of cell k: k*C*128 + j
        slot = np.arange(len(et)) - starts[key] + key * C * 128
        idx[slot] = ei.astype(np.int16)
        dlc[slot] = ep.astype(np.float32)
        # wrap idx: position i -> [i%16, i//16], replicated to 128 partitions
        idx = idx.reshape(NT * NBUK * C, 128)  # per chunk? no: per cell C*128
        idx = idx.reshape(NT * NBUK, C * 128)
        S = C * 128 // 16
        wrapped = np.zeros((16, NT * NBUK * S), np.int16)
        for cell in range(NT * NBUK):
            flat = idx[cell]
            wrapped[:, cell * S : (cell + 1) * S] = flat.reshape(S, 16).T
        idxT = np.tile(wrapped, (8, 1))  # [128, NT*4*S]
        # dlocal: [128, NT*4*C]; column (cell*C + j), partition p
        dlT = dlc.reshape(NT * NBUK * C, 128).T.copy()  # [128, cells*C]
        cores.append(dict(idxT=idxT, dlT=dlT))

    return dict(
        C=C,
        dinv=dinv,
        tile_of=tile_of,
        pos_of=pos_of,
        table_row=table_row,
        cores=cores,
    )


# ------------------------------------------------------------- device build

_BUILD_CACHE = {}


def _build(C, nsteps):
    key = (C, nsteps)
    if key in _BUILD_CACHE:
        return _BUILD_CACHE[key]
    S = C * 128 // 16

    nc = bacc.Bacc(
        "TRN2",
        target_bir_lowering=False,
        debug=False,
        num_devices=W,
        num_swdge_queues=4,
    )
    f32 = mybir.dt.float32
    xT = nc.dram_tensor("xT", [128, ROWS], f32, kind="ExternalInput")
    w1 = nc.dram_tensor("w1", [IN_DIM, HIDDEN], f32, kind="ExternalInput")
    b1c = nc.dram_tensor("b1c", [128, 2], f32, kind="ExternalInput")
    w2 = nc.dram_tensor("w2", [HIDDEN, EMBED], f32, kind="ExternalInput")
    b2t = nc.dram_tensor("b2t", [128, EMBED], f32, kind="ExternalInput")
    iotat = nc.dram_tensor("iotat", [128, 128], f32, kind="ExternalInput")
    idx_in = nc.dram_tensor("idx_in", [128, NT * NBUK * S], mybir.dt.int16, kind="ExternalInput")
    dl_in = nc.dram_tensor("dl_in", [128, NT * NBUK * C], f32, kind="ExternalInput")
    dinv1a_in = nc.dram_tensor("dinv1a", [128, NT], f32, kind="ExternalInput")
    dinvp_in = nc.dram_tensor("dinvp", [128, NT], f32, kind="ExternalInput")
    hout = nc.dram_tensor("hout", [ROWS, EMBED], f32, kind="ExternalOutput")

    pshard = nc.dram_tensor("pshard", [ROWS, EMBED], f32, kind="Internal")
    tableT = nc.dram_tensor(
        "tableT", [ROWS * W, EMBED], f32, kind="Internal", addr_space="Shared"
    )
    rg = [list(range(W))]

    nc.gpsimd.load_library(mlp_lib)

    ctx = contextlib.ExitStack()
    with tile.TileContext(nc) as tc:
        with (
            tc.tile_pool(name="const", bufs=1) as cp,
            tc.tile_pool(name="xt", bufs=3) as xp,
            tc.tile_pool(name="h1", bufs=2) as h1p,
            tc.tile_pool(name="stage", bufs=3) as stp,
            tc.tile_pool(name="sel", bufs=6) as selp,
            tc.tile_pool(name="epi", bufs=3) as epp,
            tc.tile_pool(name="pm1", bufs=2, space="PSUM") as pm1,
            tc.tile_pool(name="pm2", bufs=2, space="PSUM") as pm2,
            tc.tile_pool(name="pagg", bufs=4, space="PSUM") as pagg,
        ):
            # resident constants
            idxt = cp.tile([128, NT * NBUK * S], mybir.dt.int16)
            nc.sync.dma_start(out=idxt[:], in_=idx_in[:])
            dlt = cp.tile([128, NT * NBUK * C], f32)
            nc.sync.dma_start(out=dlt[:], in_=dl_in[:])
            iot = cp.tile([128, 128], f32)
            nc.sync.dma_start(out=iot[:], in_=iotat[:])
            w1t = cp.tile([128, HIDDEN], f32)
            nc.sync.dma_start(out=w1t[:], in_=w1[:])
            b1t = cp.tile([128, 2], f32)
            nc.sync.dma_start(out=b1t[:], in_=b1c[:])
            w2t = cp.tile([128, 2, EMBED], f32)
            nc.sync.dma_start(out=w2t[:, 0, :], in_=w2[0:128, :])
            nc.sync.dma_start(out=w2t[:, 1, :], in_=w2[128:256, :])
            b2tt = cp.tile([128, EMBED], f32)
            nc.sync.dma_start(out=b2tt[:], in_=b2t[:])
            d1a = cp.tile([128, NT], f32)
            nc.sync.dma_start(out=d1a[:], in_=dinv1a_in[:])
            dpv = cp.tile([128, NT], f32)
            nc.sync.dma_start(out=dpv[:], in_=dinvp_in[:])
            x0s = cp.tile([128, NT * EMBED], f32)  # alpha * x0, tile-major

            # ---------------- MLP ----------------
            for t in range(NT):
                xt = xp.tile([128, 128], f32, tag="xt")
                nc.sync.dma_start(out=xt[:], in_=xT[:, t * 128 : (t + 1) * 128])
                ha = h1p.tile([128, 128], f32, tag="ha")
                hb = h1p.tile([128, 128], f32, tag="hb")
                p1 = pm1.tile([128, 128], f32, tag="p1")
                nc.tensor.matmul(p1[:], w1t[:, 0:128], xt[:], start=True, stop=True)
                nc.scalar.activation(
                    ha[:], p1[:], mybir.ActivationFunctionType.Relu, bias=b1t[:, 0:1]
                )
                p1b = pm1.tile([128, 128], f32, tag="p1")
                nc.tensor.matmul(p1b[:], w1t[:, 128:256], xt[:], start=True, stop=True)
                nc.scalar.activation(
                    hb[:], p1b[:], mybir.ActivationFunctionType.Relu, bias=b1t[:, 1:2]
                )
                p2 = pm2.tile([128, EMBED], f32, tag="p2")
                nc.tensor.matmul(p2[:], ha[:], w2t[:, 0, :], start=True, stop=False)
                nc.tensor.matmul(p2[:], hb[:], w2t[:, 1, :], start=False, stop=True)
                h0 = epp.tile([128, EMBED], f32, tag="h0")
                nc.vector.tensor_tensor(
                    out=h0[:], in0=p2[:], in1=b2tt[:], op=mybir.AluOpType.add
                )
                nc.vector.tensor_scalar_mul(
                    x0s[:, t * EMBED : (t + 1) * EMBED], h0[:], ALPHA
                )
                p0 = epp.tile([128, EMBED], f32, tag="p0")
                nc.vector.tensor_scalar_mul(p0[:], h0[:], dpv[:, t : t + 1])
                nc.sync.dma_start(
                    out=pshard[t * 128 : (t + 1) * 128, :], in_=p0[:]
                )

            nc.gpsimd.collective_compute(
                "AllGather", mybir.AluOpType.bypass,
                replica_groups=rg, ins=[pshard[:]], outs=[tableT[:]],
            )

            # ---------------- propagation steps ----------------
            for k in range(nsteps):
                last = k == nsteps - 1
                for t in range(NT):
                    pprev = epp.tile([128, EMBED], f32, tag="pprev")
                    nc.sync.dma_start(
                        out=pprev[:], in_=pshard[t * 128 : (t + 1) * 128, :]
                    )
                    stg = []
                    for b in range(NBUK):
                        st = stp.tile([128, C, EMBED], f32, tag=f"st{b}")
                        cell = t * NBUK + b
                        nc.gpsimd.dma_gather(
                            out_ap=st[:],
                            in_ap=tableT[b * WIN : (b + 1) * WIN, :],
                            idxs_ap=idxt[:, cell * S : (cell + 1) * S],
                            num_idxs=C * 128,
                            num_idxs_reg=C * 128,
                            elem_size=EMBED,
                            single_packet=False,
                            queue_num=b,
                        )
                        stg.append(st)
                    pg = pagg.tile([128, EMBED], f32, tag="pagg")
                    nchunks = NBUK * C
                    ci = 0
                    for b in range(NBUK):
                        for j in range(C):
                            col = (t * NBUK + b) * C + j
                            sel = selp.tile([128, 128], f32, tag="sel")
                            nc.vector.tensor_tensor(
                                out=sel[:],
                                in0=iot[:],
                                in1=dlt[:, col : col + 1].to_broadcast([128, 128]),
                                op=mybir.AluOpType.is_equal,
                            )
                            nc.tensor.matmul(
                                pg[:], sel[:], stg[b][:, j, :],
                                start=(ci == 0), stop=(ci == nchunks - 1),
                            )
                            ci += 1
                    ht = epp.tile([128, EMBED], f32, tag="ht")
                    nc.vector.tensor_tensor(
                        out=ht[:], in0=pg[:], in1=pprev[:], op=mybir.AluOpType.add
                    )
                    nc.vector.tensor_scalar_mul(ht[:], ht[:], d1a[:, t : t + 1])
                    nc.vector.tensor_tensor(
                        out=ht[:], in0=ht[:],
                        in1=x0s[:, t * EMBED : (t + 1) * EMBED],
                        op=mybir.AluOpType.add,
                    )
                    if last:
                        nc.sync.dma_start(
                            out=hout[t * 128 : (t + 1) * 128, :], in_=ht[:]
                        )
                    else:
                        pt = epp.tile([128, EMBED], f32, tag="pt")
                        nc.vector.tensor_scalar_mul(pt[:], ht[:], dpv[:, t : t + 1])
                        nc.sync.dma_start(
                            out=pshard[t * 128 : (t + 1) * 128, :], in_=pt[:]
                        )
                if not last:
                    nc.gpsimd.collective_compute(
                        "AllGather", mybir.AluOpType.bypass,
                        replica_groups=rg, ins=[pshard[:]], outs=[tableT[:]],
                    )

    nc.compile()
    ctx.close()
    _BUILD_CACHE[key] = nc
    return nc


# ------------------------------------------------------------------- driver

def kernel(x, edge_index, W1, b1, W2, b2):
    global LAST_EXEC_TIME_NS
    x = np.asarray(x, np.float32)
    W1 = np.asarray(W1, np.float32)
    b1 = np.asarray(b1, np.float32)
    W2 = np.asarray(W2, np.float32)
    b2 = np.asarray(b2, np.float32)

    prep = _prep(edge_index)
    C = prep["C"]
    nc = _build(C, K_STEPS)

    dinv = prep["dinv"]
    tile_of, pos_of = prep["tile_of"], prep["pos_of"]
    iota = np.tile(np.arange(128, dtype=np.float32)[None, :], (128, 1))
    b1c = b1.reshape(2, 128).T.copy()  # b1c[p, half] = b1[128*half + p]
    b2t = np.tile(b2[None, :], (128, 1)).astype(np.float32)

    in_maps = []
    for c in range(W):
        lo = c * PER
        ids = np.arange(lo, lo + PER)
        rowpos = 128 * tile_of[ids] + pos_of[ids]  # position within shard
        xT = np.zeros((128, ROWS), np.float32)
        # xT[:, rowpos] = x[ids].T ; shard row r holds node with rowpos==r
        xT[:, rowpos] = x[ids].T
        dvec = np.zeros(ROWS, np.float32)
        dvec[rowpos] = dinv[ids]
        dinv1a = ((1.0 - ALPHA) * dvec).reshape(NT, 128).T.copy()
        dinvp = dvec.reshape(NT, 128).T.copy()
        cc = prep["cores"][c]
        in_maps.append(
            {
                "xT": xT,
                "w1": W1,
                "b1c": b1c,
                "w2": W2,
                "b2t": b2t,
                "iotat": iota,
                "idx_in": cc["idxT"],
                "dl_in": cc["dlT"],
                "dinv1a": dinv1a,
                "dinvp": dinvp,
            }
        )

    res = bass_utils.run_bass_kernel_spmd(
        nc,
        in_maps,
        core_ids=list(range(W)),
        trace=bool(int(os.environ.get("APPNP_TRACE", "0"))),
    )
    LAST_EXEC_TIME_NS = res.exec_time_ns

    out = np.empty((N_NODES, EMBED), np.float32)
    for c in range(W):
        lo = c * PER
        ids = np.arange(lo, lo + PER)
        rowpos = 128 * tile_of[ids] + pos_of[ids]
        out[ids] = res.results[c]["hout"][rowpos]
    return out
